# revision 4
# baseline (speedup 1.0000x reference)
"""Distributed Trainium2 kernel for nn_ContrastiveLoss (SimCLR InfoNCE loss).

Math (matches the JAX reference):
    cos = (z/||z||) @ (z/||z||)^T          # [N, N], N=8192, D=1024
    logits = cos / T  (T=0.1), diag masked (exp -> 0)
    nll_i = -logits[i, (i+N/2) mod N] + log(sum_j exp(logits[i, j]))
    out = mean(nll)

Key optimizations vs the v1 full-slab kernel:
  1. Symmetry: cos is symmetric, so only the upper triangle of the 16x16
     grid of 512x512 blocks is computed (136 blocks vs 256). Round-robin
     (circular tournament) decomposition makes the schedule core-uniform:
     core c owns block-rows {2c, 2c+1}; for each it computes the diagonal
     block (d=0) and cyclic-offset blocks d=1..7, plus one d=8 block
     (c, c+8) that also carries the positive pairs. 17 blocks per core,
     every unordered block pair covered exactly once.
  2. fp8 (e4m3) inputs with DoubleRow matmuls: K=256 per MM, ~1.7x PE
     throughput vs f32r. z-hat is scaled by 16 before quantization so
     elements sit in fp8's normal range; exp() scale absorbs the 1/256.
  3. Each block's exp() values serve BOTH triangles: row-sums via DVE
     reduce (for rows of block-row I) and column-sums via a cheap
     ones-vector DoubleRow matmul into PSUM (for rows of block-col J).
  4. Column-rolled per-core input (as in v1) keeps the program static
     across cores; the d=8 slabs ride in via a small side tensor.

Host assembles: rowsum_total[i] (f64) from row-sum and col-sum partials,
nll_i = ln(rowsum_i) - (10/256) * posdot_i, output = mean (f32).
"""

import numpy as np

N, D = 8192, 1024
NCORES = 8
G = 16              # block grid (G x G blocks of BS x BS)
BS = 512            # block size
MT = 4              # m-tiles (128 rows) per block
KP = 4              # k-pairs (256-deep DoubleRow contraction steps)
NSLAB = 9           # local rolled slabs needed per core
SCL = 16.0          # pre-quantization scale on z-hat
ACT_SCALE = 10.0 / (SCL * SCL)   # (1/T) / SCL^2
BIG = 65536.0       # diag mask subtracted pre-exp (exp -> 0 exactly)
NUP = 15            # non-diagonal (colsum-producing) slots per core
WARMUP_MM = 0


def slot_table():
    """Static per-core schedule. Entries: (lhs_src, lhs_idx, rhs_idx, kind).
    lhs_src 'z': lhs slab = local slab lhs_idx (0/1), rhs = local slab
    rhs_idx of the rolled tensor. 'p': slabs 0/1 of the side tensor pp.
    Local slab r maps to global block-col (2c + r) % 16."""
    slots = [("z", 0, 0, "diag"), ("z", 1, 1, "diag")]
    for d in range(1, 8):
        slots.append(("z", 0, d, "up"))
        slots.append(("z", 1, 1 + d, "up"))
    slots.append(("p", 0, 1, "pos"))
    return slots


def _import_concourse():
    import sys
    try:
        import concourse.bass  # noqa: F401
    except ImportError:
        for p in ("/root/.axon_site/_ro/trn_rl_repo", "/opt/trn_rl_repo"):
            if p not in sys.path:
                sys.path.insert(0, p)
        import concourse.bass  # noqa: F401


def build_program():
    _import_concourse()
    import concourse.mybir as mybir
    import concourse.tile as tile
    from concourse import bacc
    from concourse.masks import make_identity

    f32 = mybir.dt.float32
    f8 = mybir.dt.float8e4
    Act = mybir.ActivationFunctionType
    DR = mybir.MatmulPerfMode.DoubleRow
    X = mybir.AxisListType.X

    slots = slot_table()

    nc = bacc.Bacc()
    zc = nc.declare_dram_parameter("zc", [D, NSLAB * BS], f8, isOutput=False)
    pp = nc.declare_dram_parameter("pp", [D, 2 * BS], f8, isOutput=False)
    rs_out = nc.declare_dram_parameter("rs_out", [128, 17 * MT], f32, isOutput=True)
    cs_out = nc.declare_dram_parameter("cs_out", [1, NUP * BS], f32, isOutput=True)
    pc_out = nc.declare_dram_parameter("pc_out", [128, MT], f32, isOutput=True)

    zc_pkn = zc.rearrange("(k p) n -> p k n", p=128)
    pp_pkn = pp.rearrange("(k p) n -> p k n", p=128)

    with tile.TileContext(nc) as tc:
        with (
            tc.tile_pool(name="consts", bufs=1) as consts,
            tc.tile_pool(name="zpool", bufs=1) as zpool,
            tc.tile_pool(name="epool", bufs=4) as epool,
            tc.tile_pool(name="dpool", bufs=2) as dpool,
            tc.tile_pool(name="accp", bufs=1) as accp,
            tc.tile_pool(name="psump", bufs=6, space="PSUM") as psump,
            tc.tile_pool(name="cspp", bufs=2, space="PSUM") as cspp,
        ):
            zsb = zpool.tile([128, 8, NSLAB * BS], f8)
            ppsb = zpool.tile([128, 8, 2 * BS], f8)
            # one DMA per slab (all 8 k at once); first blocks touch only
            # early slabs, so compute starts after ~512 KB lands.
            for s in range(NSLAB):
                nc.sync.dma_start(
                    out=zsb[:, :, s * BS : (s + 1) * BS],
                    in_=zc_pkn[:, :, s * BS : (s + 1) * BS],
                )
            nc.sync.dma_start(out=ppsb, in_=pp_pkn)

            ident = consts.tile([128, 128], f32)
            make_identity(nc, ident)
            identw = consts.tile([128, 1], f32)
            nc.vector.reduce_max(out=identw, in_=ident, axis=X)
            identbig = consts.tile([128, 128], f32)
            nc.vector.tensor_scalar_mul(out=identbig, in0=ident, scalar1=BIG)
            ones2 = consts.tile([128, 2, 16], f8)
            nc.gpsimd.memset(ones2, 1.0)

            rssb = accp.tile([128, 17 * MT], f32)
            pcsb = accp.tile([128, MT], f32)
            cssb = accp.tile([1, NUP * BS], f32)

            # No HAM warmup: f32 ident matmuls lower to LOW_HIGH two-pass
            # mode (~420ns each) and occupy the PE queue until ~12us,
            # delaying the real fp8 stream past the first slab's DMA.
            if WARMUP_MM:
                wps = psump.tile([128, 512], f32, tag="ps")
                for _ in range(WARMUP_MM):
                    nc.tensor.matmul(
                        wps[:, :128], lhsT=ident, rhs=ident,
                        start=True, stop=True,
                    )

            # software-pipelined colsum MMs: emit one pair late so the PE
            # never waits on the ACT engine's exp of the current pair.
            pending_cs = []

            def flush_cs(n_keep):
                while len(pending_cs) > n_keep:
                    args = pending_cs.pop(0)
                    nc.tensor.matmul(**args, skip_group_check=True)

            csidx = 0
            cs_tiles = []
            for s, (lsrc, li, ri, kind) in enumerate(slots):
                lbase = zsb if lsrc == "z" else ppsb
                rbase = zsb if lsrc == "z" else ppsb
                lo = li * BS
                ro = ri * BS
                csps = None
                if kind != "diag":
                    csps = cspp.tile([128, BS], f32, tag="cs")
                for pair in range(2):
                    E = epool.tile([128, 2, BS], f8)
                    for sub in range(2):
                        m = pair * 2 + sub
                        ps = psump.tile([128, BS], f32, tag="ps")
                        for kp in range(KP):
                            nc.tensor.matmul(
                                ps,
                                lhsT=lbase[:, 2 * kp : 2 * kp + 2,
                                           lo + m * 128 : lo + (m + 1) * 128],
                                rhs=rbase[:, 2 * kp : 2 * kp + 2, ro : ro + BS],
                                start=(kp == 0),
                                stop=(kp == KP - 1),
                                perf_mode=DR,
                            )
                        # emit delayed colsum MMs after this m-tile's mains
                        flush_cs(1)
                        if kind == "diag":
                            sl = ps[:, m * 128 : (m + 1) * 128]
                            nc.vector.tensor_sub(out=sl, in0=sl, in1=identbig)
                        if kind == "pos":
                            dt = dpool.tile([128, 128], f32)
                            nc.vector.tensor_mul(
                                out=dt, in0=ps[:, m * 128 : (m + 1) * 128],
                                in1=ident,
                            )
                            nc.vector.reduce_sum(
                                out=pcsb[:, m : m + 1], in_=dt, axis=X
                            )
                        nc.scalar.activation(
                            out=E[:, sub, :], in_=ps, func=Act.Exp,
                            scale=ACT_SCALE,
                        )
                    c0 = s * MT + pair * 2
                    nc.vector.reduce_sum(
                        out=rssb[:, c0 : c0 + 2], in_=E, axis=X
                    )
                    if kind != "diag":
                        pending_cs.append(dict(
                            out=csps[0:1, :],
                            lhsT=ones2[:, :, 0:1],
                            rhs=E[:, :, :],
                            start=(pair == 0),
                            stop=(pair == 1),
                            perf_mode=DR,
                        ))
                if kind != "diag":
                    cs_tiles.append((csidx, csps))
                    csidx += 1
                # drain finished colsum PSUM banks to SBUF (ACT copy)
                while cs_tiles and len(cs_tiles) > 1:
                    i, t = cs_tiles.pop(0)
                    nc.scalar.activation(
                        out=cssb[0:1, i * BS : (i + 1) * BS],
                        in_=t[0:1, :], func=Act.Copy,
                    )
            flush_cs(0)
            for i, t in cs_tiles:
                nc.scalar.activation(
                    out=cssb[0:1, i * BS : (i + 1) * BS],
                    in_=t[0:1, :], func=Act.Copy,
                )

            nc.sync.dma_start(out=cs_out[:, :], in_=cssb)
            nc.sync.dma_start(out=rs_out[:, :], in_=rssb)
            nc.sync.dma_start(out=pc_out[:, :], in_=pcsb)
    nc.finalize()
    return nc


def make_in_maps(z: np.ndarray) -> list[dict]:
    import ml_dtypes

    z64 = np.asarray(z, np.float64)
    zn = z64 / np.linalg.norm(z64, axis=1, keepdims=True)
    zq = (SCL * zn).astype(ml_dtypes.float8_e4m3)        # [N, D]
    zqT = np.ascontiguousarray(zq.T)                     # [D, N]
    in_maps = []
    for c in range(NCORES):
        r = (2 * c * BS) % N
        zroll = np.concatenate([zqT[:, r:], zqT[:, :r]], axis=1)[:, : NSLAB * BS]
        ppc = np.concatenate(
            [zqT[:, c * BS : (c + 1) * BS],
             zqT[:, (c + 8) * BS : (c + 9) * BS]], axis=1)
        in_maps.append({
            "zc": np.ascontiguousarray(zroll),
            "pp": np.ascontiguousarray(ppc),
        })
    return in_maps


def assemble(results: list[dict]) -> np.ndarray:
    slots = slot_table()
    rowsum = np.zeros(N, np.float64)
    pos = np.zeros(N, np.float64)
    for c, res in enumerate(results):
        rs = np.asarray(res["rs_out"], np.float64)   # [128, 68]
        cs = np.asarray(res["cs_out"], np.float64).reshape(NUP, BS)
        pc = np.asarray(res["pc_out"], np.float64)   # [128, 4]
        csi = 0
        for s, (lsrc, li, ri, kind) in enumerate(slots):
            I = (2 * c + li) % G if lsrc == "z" else c
            for m in range(MT):
                rowsum[I * BS + m * 128 : I * BS + (m + 1) * 128] += rs[:, s * MT + m]
            if kind != "diag":
                J = (2 * c + ri) % G if lsrc == "z" else c + 8
                rowsum[J * BS : (J + 1) * BS] += cs[csi]
                csi += 1
        for m in range(MT):
            pos[c * BS + m * 128 : c * BS + (m + 1) * 128] = pc[:, m]
            pos[(c + 8) * BS + m * 128 : (c + 8) * BS + (m + 1) * 128] = pc[:, m]
    nll = np.log(rowsum) - pos * (10.0 / (SCL * SCL))
    return np.float32(nll.mean())


def kernel(z: np.ndarray) -> np.ndarray:
    _import_concourse()
    from concourse.bass_utils import run_bass_kernel_spmd

    nc = build_program()
    in_maps = make_in_maps(z)
    res = run_bass_kernel_spmd(nc, in_maps, core_ids=list(range(NCORES)))
    return assemble(res.results)


# revision 5
# speedup vs baseline: 1.0285x; 1.0285x over previous
"""Distributed Trainium2 kernel for nn_ContrastiveLoss (SimCLR InfoNCE loss).

Math (matches the JAX reference):
    cos = (z/||z||) @ (z/||z||)^T          # [N, N], N=8192, D=1024
    logits = cos / T  (T=0.1), diag masked (exp -> 0)
    nll_i = -logits[i, (i+N/2) mod N] + log(sum_j exp(logits[i, j]))
    out = mean(nll)

Key optimizations vs the v1 full-slab kernel:
  1. Symmetry: cos is symmetric, so only the upper triangle of the 16x16
     grid of 512x512 blocks is computed (136 blocks vs 256). Round-robin
     (circular tournament) decomposition makes the schedule core-uniform:
     core c owns block-rows {2c, 2c+1}; for each it computes the diagonal
     block (d=0) and cyclic-offset blocks d=1..7, plus one d=8 block
     (c, c+8) that also carries the positive pairs. 17 blocks per core,
     every unordered block pair covered exactly once.
  2. fp8 (e4m3) inputs with DoubleRow matmuls: K=256 per MM, ~1.7x PE
     throughput vs f32r. z-hat is scaled by 16 before quantization so
     elements sit in fp8's normal range; exp() scale absorbs the 1/256.
  3. Each block's exp() values serve BOTH triangles: row-sums via DVE
     reduce (for rows of block-row I) and column-sums via a cheap
     ones-vector DoubleRow matmul into PSUM (for rows of block-col J).
  4. Column-rolled per-core input (as in v1) keeps the program static
     across cores; the d=8 slabs ride in via a small side tensor.

Host assembles: rowsum_total[i] (f64) from row-sum and col-sum partials,
nll_i = ln(rowsum_i) - (10/256) * posdot_i, output = mean (f32).
"""

import numpy as np

N, D = 8192, 1024
NCORES = 8
G = 16              # block grid (G x G blocks of BS x BS)
BS = 512            # block size
MT = 4              # m-tiles (128 rows) per block
KP = 4              # k-pairs (256-deep DoubleRow contraction steps)
NSLAB = 9           # local rolled slabs needed per core
SCL = 16.0          # pre-quantization scale on z-hat
ACT_SCALE = 10.0 / (SCL * SCL)   # (1/T) / SCL^2
BIG = 65536.0       # diag mask subtracted pre-exp (exp -> 0 exactly)
NUP = 15            # non-diagonal (colsum-producing) slots per core
WARMUP_MM = 10


def slot_table():
    """Static per-core schedule. Entries: (lhs_src, lhs_idx, rhs_idx, kind).
    lhs_src 'z': lhs slab = local slab lhs_idx (0/1), rhs = local slab
    rhs_idx of the rolled tensor. 'p': slabs 0/1 of the side tensor pp.
    Local slab r maps to global block-col (2c + r) % 16."""
    slots = [("z", 0, 0, "diag"), ("z", 1, 1, "diag")]
    for d in range(1, 8):
        slots.append(("z", 0, d, "up"))
        slots.append(("z", 1, 1 + d, "up"))
    slots.append(("p", 0, 1, "pos"))
    return slots


def _import_concourse():
    import sys
    try:
        import concourse.bass  # noqa: F401
    except ImportError:
        for p in ("/root/.axon_site/_ro/trn_rl_repo", "/opt/trn_rl_repo"):
            if p not in sys.path:
                sys.path.insert(0, p)
        import concourse.bass  # noqa: F401


def build_program():
    _import_concourse()
    import concourse.mybir as mybir
    import concourse.tile as tile
    from concourse import bacc
    from concourse.masks import make_identity

    f32 = mybir.dt.float32
    f8 = mybir.dt.float8e4
    Act = mybir.ActivationFunctionType
    DR = mybir.MatmulPerfMode.DoubleRow
    X = mybir.AxisListType.X

    slots = slot_table()

    nc = bacc.Bacc()
    zc = nc.declare_dram_parameter("zc", [D, NSLAB * BS], f8, isOutput=False)
    pp = nc.declare_dram_parameter("pp", [D, 2 * BS], f8, isOutput=False)
    rs_out = nc.declare_dram_parameter("rs_out", [128, 17 * MT], f32, isOutput=True)
    cs_out = nc.declare_dram_parameter("cs_out", [1, NUP * BS], f32, isOutput=True)
    pc_out = nc.declare_dram_parameter("pc_out", [128, MT], f32, isOutput=True)

    zc_pkn = zc.rearrange("(k p) n -> p k n", p=128)
    pp_pkn = pp.rearrange("(k p) n -> p k n", p=128)

    with tile.TileContext(nc) as tc:
        with (
            tc.tile_pool(name="consts", bufs=1) as consts,
            tc.tile_pool(name="zpool", bufs=1) as zpool,
            tc.tile_pool(name="epool", bufs=4) as epool,
            tc.tile_pool(name="dpool", bufs=2) as dpool,
            tc.tile_pool(name="accp", bufs=1) as accp,
            tc.tile_pool(name="psump", bufs=6, space="PSUM") as psump,
            tc.tile_pool(name="cspp", bufs=2, space="PSUM") as cspp,
        ):
            zsb = zpool.tile([128, 8, NSLAB * BS], f8)
            ppsb = zpool.tile([128, 8, 2 * BS], f8)
            # one DMA per slab (all 8 k at once); first blocks touch only
            # early slabs, so compute starts after ~512 KB lands.
            for s in range(NSLAB):
                nc.sync.dma_start(
                    out=zsb[:, :, s * BS : (s + 1) * BS],
                    in_=zc_pkn[:, :, s * BS : (s + 1) * BS],
                )
            nc.sync.dma_start(out=ppsb, in_=pp_pkn)

            ident = consts.tile([128, 128], f32)
            make_identity(nc, ident)
            identw = consts.tile([128, 1], f32)
            nc.vector.reduce_max(out=identw, in_=ident, axis=X)
            identbig = consts.tile([128, 128], f32)
            nc.vector.tensor_scalar_mul(out=identbig, in0=ident, scalar1=BIG)
            ones2 = consts.tile([128, 2, 16], f8)
            nc.gpsimd.memset(ones2, 1.0)

            rssb = accp.tile([128, 17 * MT], f32)
            pcsb = accp.tile([128, MT], f32)
            cssb = accp.tile([1, NUP * BS], f32)

            # No HAM warmup: f32 ident matmuls lower to LOW_HIGH two-pass
            # mode (~420ns each) and occupy the PE queue until ~12us,
            # delaying the real fp8 stream past the first slab's DMA.
            if WARMUP_MM:
                wps = psump.tile([128, 512], f32, tag="ps")
                for _ in range(WARMUP_MM):
                    nc.tensor.matmul(
                        wps[:, :128], lhsT=ident, rhs=ident,
                        start=True, stop=True,
                    )

            # software-pipelined colsum MMs: emit one pair late so the PE
            # never waits on the ACT engine's exp of the current pair.
            pending_cs = []

            def flush_cs(n_keep):
                while len(pending_cs) > n_keep:
                    args = pending_cs.pop(0)
                    nc.tensor.matmul(**args, skip_group_check=True)

            csidx = 0
            cs_tiles = []
            for s, (lsrc, li, ri, kind) in enumerate(slots):
                lbase = zsb if lsrc == "z" else ppsb
                rbase = zsb if lsrc == "z" else ppsb
                lo = li * BS
                ro = ri * BS
                csps = None
                if kind != "diag":
                    csps = cspp.tile([128, BS], f32, tag="cs")
                for pair in range(2):
                    E = epool.tile([128, 2, BS], f8)
                    for sub in range(2):
                        m = pair * 2 + sub
                        ps = psump.tile([128, BS], f32, tag="ps")
                        for kp in range(KP):
                            nc.tensor.matmul(
                                ps,
                                lhsT=lbase[:, 2 * kp : 2 * kp + 2,
                                           lo + m * 128 : lo + (m + 1) * 128],
                                rhs=rbase[:, 2 * kp : 2 * kp + 2, ro : ro + BS],
                                start=(kp == 0),
                                stop=(kp == KP - 1),
                                perf_mode=DR,
                            )
                        # emit delayed colsum MMs after this m-tile's mains
                        flush_cs(1)
                        if kind == "diag":
                            sl = ps[:, m * 128 : (m + 1) * 128]
                            nc.vector.tensor_sub(out=sl, in0=sl, in1=identbig)
                        if kind == "pos":
                            dt = dpool.tile([128, 128], f32)
                            nc.vector.tensor_mul(
                                out=dt, in0=ps[:, m * 128 : (m + 1) * 128],
                                in1=ident,
                            )
                            nc.vector.reduce_sum(
                                out=pcsb[:, m : m + 1], in_=dt, axis=X
                            )
                        nc.scalar.activation(
                            out=E[:, sub, :], in_=ps, func=Act.Exp,
                            scale=ACT_SCALE,
                        )
                    c0 = s * MT + pair * 2
                    nc.vector.reduce_sum(
                        out=rssb[:, c0 : c0 + 2], in_=E, axis=X
                    )
                    if kind != "diag":
                        pending_cs.append(dict(
                            out=csps[0:1, :],
                            lhsT=ones2[:, :, 0:1],
                            rhs=E[:, :, :],
                            start=(pair == 0),
                            stop=(pair == 1),
                            perf_mode=DR,
                        ))
                if kind != "diag":
                    cs_tiles.append((csidx, csps))
                    csidx += 1
                # drain finished colsum PSUM banks to SBUF (ACT copy)
                while cs_tiles and len(cs_tiles) > 1:
                    i, t = cs_tiles.pop(0)
                    nc.scalar.activation(
                        out=cssb[0:1, i * BS : (i + 1) * BS],
                        in_=t[0:1, :], func=Act.Copy,
                    )
            flush_cs(0)
            for i, t in cs_tiles:
                nc.scalar.activation(
                    out=cssb[0:1, i * BS : (i + 1) * BS],
                    in_=t[0:1, :], func=Act.Copy,
                )

            nc.sync.dma_start(out=cs_out[:, :], in_=cssb)
            nc.sync.dma_start(out=rs_out[:, :], in_=rssb)
            nc.sync.dma_start(out=pc_out[:, :], in_=pcsb)
    nc.finalize()
    return nc


def make_in_maps(z: np.ndarray) -> list[dict]:
    import ml_dtypes

    z64 = np.asarray(z, np.float64)
    zn = z64 / np.linalg.norm(z64, axis=1, keepdims=True)
    zq = (SCL * zn).astype(ml_dtypes.float8_e4m3)        # [N, D]
    zqT = np.ascontiguousarray(zq.T)                     # [D, N]
    in_maps = []
    for c in range(NCORES):
        r = (2 * c * BS) % N
        zroll = np.concatenate([zqT[:, r:], zqT[:, :r]], axis=1)[:, : NSLAB * BS]
        ppc = np.concatenate(
            [zqT[:, c * BS : (c + 1) * BS],
             zqT[:, (c + 8) * BS : (c + 9) * BS]], axis=1)
        in_maps.append({
            "zc": np.ascontiguousarray(zroll),
            "pp": np.ascontiguousarray(ppc),
        })
    return in_maps


def assemble(results: list[dict]) -> np.ndarray:
    slots = slot_table()
    rowsum = np.zeros(N, np.float64)
    pos = np.zeros(N, np.float64)
    for c, res in enumerate(results):
        rs = np.asarray(res["rs_out"], np.float64)   # [128, 68]
        cs = np.asarray(res["cs_out"], np.float64).reshape(NUP, BS)
        pc = np.asarray(res["pc_out"], np.float64)   # [128, 4]
        csi = 0
        for s, (lsrc, li, ri, kind) in enumerate(slots):
            I = (2 * c + li) % G if lsrc == "z" else c
            for m in range(MT):
                rowsum[I * BS + m * 128 : I * BS + (m + 1) * 128] += rs[:, s * MT + m]
            if kind != "diag":
                J = (2 * c + ri) % G if lsrc == "z" else c + 8
                rowsum[J * BS : (J + 1) * BS] += cs[csi]
                csi += 1
        for m in range(MT):
            pos[c * BS + m * 128 : c * BS + (m + 1) * 128] = pc[:, m]
            pos[(c + 8) * BS + m * 128 : (c + 8) * BS + (m + 1) * 128] = pc[:, m]
    nll = np.log(rowsum) - pos * (10.0 / (SCL * SCL))
    return np.float32(nll.mean())


def kernel(z: np.ndarray) -> np.ndarray:
    _import_concourse()
    from concourse.bass_utils import run_bass_kernel_spmd

    nc = build_program()
    in_maps = make_in_maps(z)
    res = run_bass_kernel_spmd(nc, in_maps, core_ids=list(range(NCORES)))
    return assemble(res.results)
